# revision 1
# baseline (speedup 1.0000x reference)
"""Trainium2 Bass kernel for nn_DecoderLayer (RNMT+ LN-LSTM decoder layer).

Two-stage pipeline, all device-resident between stages:

  stage A (jax/XLA, shard_map over 8 cores):
    - inputs uploaded as bf16 shards: X batch-sharded, W gate-column-sharded
      (8x less W upload than replication)
    - all_gather W columns, pre-phase GEMM  Z = [x,attn] @ Wx + b  (bf16, fp32
      accumulate), pack Z into the per-step layout the loop kernel wants
  stage B (Bass custom call, per core, batch-data-parallel BL=4):
    - the 256-step recurrence: g = z_t + h @ Wh; joint LayerNorm over the
      (4,1024) gate slab (bn_stats + indicator-matmul partition combine +
      fast-inverse-sqrt on DVE - no ACT table switches); sigmoid/tanh fused
      with the normalize via ACT scale/bias; c/h update; PE transpose of h
      for the next step's stationary operand; residual add + output.

Weights/activations bf16 (matmul), state and LN arithmetic fp32.
"""
import sys

sys.path.insert(0, "/opt/trn_rl_repo")

import numpy as np

import concourse.bass as bass
import concourse.tile as tile
from concourse import bacc, mybir

B, S, ISIZE, OSIZE = 32, 256, 1024, 1024
NCORES = 8
BL = B // NCORES  # 4 batch rows per core
INSZ = ISIZE + OSIZE  # 2048
NG = 4 * OSIZE  # 4096
EPS = 1e-5
F32, BF16, I32 = mybir.dt.float32, mybir.dt.bfloat16, mybir.dt.int32
FISR_MAGIC_F32 = float(
    np.frombuffer(np.array([0x5F3759DF], np.uint32).tobytes(), np.float32)[0]
)

ZCH = 4   # z prefetch chunk (steps per DMA)
RCH = 4   # res/out chunk (steps per DMA)

_cache = {}


def build_nc(s_steps=S, use_ln=False):
    assert s_steps % ZCH == 0 and s_steps % RCH == 0
    nc = bacc.Bacc(None)
    zd = nc.dram_tensor(
        "zd", [s_steps // ZCH, 4, BL, ZCH, OSIZE], BF16, kind="ExternalInput"
    )
    whp = nc.dram_tensor("whp", [128, 8, NG], BF16, kind="ExternalInput")
    res = nc.dram_tensor("res", [BL, s_steps, OSIZE], BF16, kind="ExternalInput")
    ihx = nc.dram_tensor("ihx", [128, 8, BL], BF16, kind="ExternalInput")
    icx = nc.dram_tensor("icx", [BL, OSIZE], F32, kind="ExternalInput")
    ind = nc.dram_tensor("ind", [128, 128], F32, kind="ExternalInput")
    if use_ln:
        lng = nc.dram_tensor("lng", [128, OSIZE], F32, kind="ExternalInput")
        lnb = nc.dram_tensor("lnb", [128, OSIZE], F32, kind="ExternalInput")
    out = nc.dram_tensor("out", [BL, s_steps, OSIZE], F32, kind="ExternalOutput")

    with tile.TileContext(nc) as tc:
        with (
            tc.tile_pool(name="cw", bufs=1) as cw,
            tc.tile_pool(name="zp", bufs=2) as zp,
            tc.tile_pool(name="rp", bufs=2) as rp,
            tc.tile_pool(name="ob", bufs=2) as ob,
            tc.tile_pool(name="sp", bufs=2) as sp,
            tc.tile_pool(name="hp", bufs=2) as hp,
            tc.tile_pool(name="yp", bufs=2) as yp,
            tc.tile_pool(name="htp", bufs=2) as htp,
            tc.tile_pool(name="gps", bufs=4, space="PSUM") as gps,
            tc.tile_pool(name="tps", bufs=1, space="PSUM") as tps,
            tc.tile_pool(name="yps", bufs=1, space="PSUM") as yps,
            tc.tile_pool(name="sps", bufs=1, space="PSUM") as sps,
        ):
            whs = cw.tile([128, 8, NG], BF16)
            nc.sync.dma_start(out=whs, in_=whp[:, :, :])
            inds = cw.tile([128, 128], F32)
            nc.sync.dma_start(out=inds, in_=ind[:, :])
            if use_ln:
                lngs = cw.tile([128, OSIZE], F32)
                nc.sync.dma_start(out=lngs, in_=lng[:, :])
                lnbs = cw.tile([128, OSIZE], F32)
                nc.sync.dma_start(out=lnbs, in_=lnb[:, :])
            magic = cw.tile([128, 1], F32)
            nc.vector.memset(magic, FISR_MAGIC_F32)
            id4 = cw.tile([BL, BL], F32)
            from concourse.masks import make_identity

            make_identity(nc, id4)

            c = cw.tile([BL, OSIZE], F32)
            nc.sync.dma_start(out=c, in_=icx[:, :])
            hT = htp.tile([128, 8, BL], BF16, tag="hT")
            nc.sync.dma_start(out=hT, in_=ihx[:, :, :])

            for t in range(s_steps):
                tz, tr = t % ZCH, t % RCH
                if tz == 0:
                    z4 = zp.tile([128, ZCH, OSIZE], BF16, tag="z4")
                    for zg in range(4):
                        nc.sync.dma_start(
                            out=z4[32 * zg : 32 * zg + BL, :, :],
                            in_=zd[t // ZCH, zg, :, :, :],
                        )
                if tr == 0:
                    resb = rp.tile([BL, RCH, OSIZE], BF16, tag="resb")
                    nc.gpsimd.dma_start(out=resb, in_=res[:, t : t + RCH, :])
                    outb = ob.tile([BL, RCH, OSIZE], F32, tag="outb")

                # ---- g = z_t + h @ Wh;  joint LN stats over the gate slab ----
                gsb = yp.tile([128, OSIZE], F32, tag="gsb")
                if t < 2:
                    nc.vector.memset(gsb, 0.0)
                stats = sp.tile([128, 2, nc.vector.BN_STATS_DIM], F32, tag="stats")
                for h2 in range(2):
                    for g4 in range(4):
                        cs = slice(h2 * 512, h2 * 512 + 512)
                        ps = gps.tile([BL, 512], F32, tag="ps")
                        for kk in range(8):
                            nc.tensor.matmul(
                                ps,
                                hT[:, kk, :],
                                whs[:, kk, g4 * 1024 + h2 * 512 : g4 * 1024 + h2 * 512 + 512],
                                start=(kk == 0),
                                stop=(kk == 7),
                            )
                        nc.vector.tensor_add(
                            gsb[32 * g4 : 32 * g4 + BL, cs], ps, z4[32 * g4 : 32 * g4 + BL, tz, cs]
                        )
                    nc.vector.bn_stats(
                        out=stats[:, h2, :], in_=gsb[:, h2 * 512 : h2 * 512 + 512]
                    )
                mv = sp.tile([128, 2], F32, tag="mv")
                nc.vector.bn_aggr(out=mv, in_=stats)
                # mv[:,1] <- E[x^2] = var + mean^2 (per partition)
                nc.vector.scalar_tensor_tensor(
                    out=mv[:, 1:2], in0=mv[:, 0:1], scalar=mv[:, 0:1],
                    in1=mv[:, 1:2],
                    op0=mybir.AluOpType.mult, op1=mybir.AluOpType.add,
                )
                pss = sps.tile([128, 2], F32, tag="pss")
                nc.tensor.matmul(pss, inds, mv, start=True, stop=True)
                q = sp.tile([128, 2], F32, tag="q")  # [mu, E[x^2]] per row
                nc.vector.tensor_copy(q, pss)
                vh = sp.tile([128, 1], F32, tag="vh")
                # vh = mu^2 - E[x^2]  (negated variance)
                nc.vector.scalar_tensor_tensor(
                    out=vh, in0=q[:, 0:1], scalar=q[:, 0:1], in1=q[:, 1:2],
                    op0=mybir.AluOpType.mult, op1=mybir.AluOpType.subtract,
                )
                # vh = -(vh) + EPS = var + EPS
                nc.vector.tensor_scalar(
                    out=vh, in0=vh, scalar1=-1.0, scalar2=EPS,
                    op0=mybir.AluOpType.mult, op1=mybir.AluOpType.add,
                )
                # fast inverse sqrt + 1 Newton iteration -> rstd
                ish = sp.tile([128, 1], I32, tag="ish")
                nc.vector.tensor_scalar(
                    out=ish, in0=vh.bitcast(I32), scalar1=1, scalar2=None,
                    op0=mybir.AluOpType.logical_shift_right,
                )
                y0 = sp.tile([128, 1], F32, tag="y0")
                nc.vector.tensor_sub(y0.bitcast(I32), magic.bitcast(I32), ish)
                t2 = sp.tile([128, 1], F32, tag="t2")
                nc.vector.scalar_tensor_tensor(
                    out=t2, in0=y0, scalar=y0, in1=vh,
                    op0=mybir.AluOpType.mult, op1=mybir.AluOpType.mult,
                )
                nc.vector.tensor_scalar(
                    out=t2, in0=t2, scalar1=-0.5, scalar2=1.5,
                    op0=mybir.AluOpType.mult, op1=mybir.AluOpType.add,
                )
                rstd = sp.tile([128, 1], F32, tag="rstd")
                nc.vector.tensor_mul(rstd, y0, t2)
                nbias = sp.tile([128, 1], F32, tag="nbias")  # -mu*rstd
                nc.vector.tensor_scalar(
                    out=nbias, in0=rstd, scalar1=q[:, 0:1], scalar2=-1.0,
                    op0=mybir.AluOpType.mult, op1=mybir.AluOpType.mult,
                )
                # activations: sigmoid gates -> PSUM (PSUM operands are
                # exempt from the equal-base-partition DVE rule), tanh -> SBUF
                ygs = yps.tile([96, OSIZE], F32, tag="ygs")
                ygt = yp.tile([BL, OSIZE], F32, tag="ygt")
                if use_ln:
                    y2n = yp.tile([128, OSIZE], F32, tag="y2n")
                    nc.vector.tensor_scalar(
                        out=y2n, in0=gsb, scalar1=rstd, scalar2=nbias,
                        op0=mybir.AluOpType.mult, op1=mybir.AluOpType.add,
                    )
                    nc.vector.tensor_mul(y2n, y2n, lngs)
                    nc.vector.tensor_add(y2n, y2n, lnbs)
                    nc.scalar.activation(
                        out=ygs, in_=y2n[0:96, :],
                        func=mybir.ActivationFunctionType.Sigmoid,
                    )
                    nc.scalar.activation(
                        out=ygt, in_=y2n[96 : 96 + BL, :],
                        func=mybir.ActivationFunctionType.Tanh,
                    )
                else:
                    nc.scalar.activation(
                        out=ygs, in_=gsb[0:96, :],
                        func=mybir.ActivationFunctionType.Sigmoid,
                        bias=nbias[0:96, :], scale=rstd[0:96, :],
                    )
                    nc.scalar.activation(
                        out=ygt, in_=gsb[96 : 96 + BL, :],
                        func=mybir.ActivationFunctionType.Tanh,
                        bias=nbias[96 : 96 + BL, :],
                        scale=rstd[96 : 96 + BL, :],
                    )

                # keep-warm: tiny PE op mid-tail so HAM stays at full clock
                psT = tps.tile([128, 8, BL], F32, tag="psT")
                nc.tensor.transpose(psT[0:64, 0, :], ygt[:, 0:64], id4)

                # ---- state update ----
                u = hp.tile([BL, OSIZE], F32, tag="u")
                nc.vector.tensor_mul(u, ygs[0:BL, :], ygt)
                # second keep-warm, fires after u mid/late tail
                nc.tensor.transpose(psT[0:64, 1, :], u[:, 0:64], id4)
                nc.vector.tensor_mul(c, ygs[32 : 32 + BL, :], c)
                nc.vector.tensor_add(c, c, u)
                h = hp.tile([BL, OSIZE], F32, tag="h")
                for hh in range(2):
                    cs = slice(hh * 512, hh * 512 + 512)
                    nc.vector.tensor_mul(h[:, cs], ygs[64 : 64 + BL, cs], c[:, cs])
                    if t + 1 < s_steps:
                        for kk in range(4 * hh, 4 * hh + 4):
                            nc.tensor.transpose(
                                psT[:, kk, :], h[:, kk * 128 : (kk + 1) * 128], id4
                            )
                nc.gpsimd.tensor_add(outb[:, tr, :], h, resb[:, tr, :])
                if tr == RCH - 1:
                    nc.gpsimd.dma_start(
                        out=out[:, t - tr : t + 1, :], in_=outb[:, :, :]
                    )
                if t + 1 < s_steps:
                    hT = htp.tile([128, 8, BL], BF16, tag="hT")
                    nc.vector.tensor_copy(hT, psT)
    nc.finalize()
    return nc



# ---------------------------------------------------------------------------
# host prep + jax pipeline (stage A: gathers + pre-phase GEMM; stage B: bass)
# ---------------------------------------------------------------------------
import ml_dtypes

BF16NP = ml_dtypes.bfloat16


def _to_bf16(a):
    """fp32 -> bf16 with round-to-nearest-even via integer view (fast)."""
    u = np.ascontiguousarray(a, np.float32).view(np.uint32)
    r = ((u + np.uint32(0x7FFF) + ((u >> np.uint32(16)) & np.uint32(1)))
         >> np.uint32(16)).astype(np.uint16)
    return r.view(BF16NP)


_pp = np.arange(128)
IND_NP = 0.25 * (
    (_pp[:, None] % 32 == _pp[None, :] % 32) & (_pp[:, None] % 32 < 4)
).astype(np.float32)


def _build_pipeline(s_steps, use_ln):
    """Returns (run, put) where put(host arrays)->device arrays and
    run(dev)->jax out array [B, s, OSIZE]."""
    import jax
    import jax.numpy as jnp
    from jax.sharding import Mesh, PartitionSpec as P, NamedSharding
    from jax.experimental.shard_map import shard_map
    from concourse.bass2jax import (
        install_neuronx_cc_hook,
        partition_id_tensor,
        _bass_exec_p,
    )

    install_neuronx_cc_hook()
    nc = build_nc(s_steps, use_ln)

    devices = jax.devices()[:NCORES]
    mesh = Mesh(np.asarray(devices), ("c",))

    # ---- stage B: bass custom call ----
    partition_name = nc.partition_id_tensor.name if nc.partition_id_tensor else None
    in_names, out_names, out_avals = [], [], []
    for alloc in nc.m.functions[0].allocations:
        if not isinstance(alloc, mybir.MemoryLocationSet):
            continue
        name = alloc.memorylocations[0].name
        if alloc.kind == "ExternalInput":
            if name != partition_name:
                in_names.append(name)
        elif alloc.kind == "ExternalOutput":
            out_names.append(name)
            import jax.core

            out_avals.append(
                jax.core.ShapedArray(tuple(alloc.tensor_shape), mybir.dt.np(alloc.dtype))
            )
    all_in = in_names + out_names + ([partition_name] if partition_name else [])

    def _bass_body(*args):
        operands = list(args)
        if partition_name is not None:
            operands.append(partition_id_tensor())
        outs = _bass_exec_p.bind(
            *operands,
            out_avals=tuple(out_avals),
            in_names=tuple(all_in),
            out_names=tuple(out_names),
            lowering_input_output_aliases=(),
            sim_require_finite=True,
            sim_require_nnan=True,
            nc=nc,
        )
        return tuple(outs)

    # bass input order: zd, whp, res, ihx, icx, ind, [lng, lnb] then out-zeros
    n_bass_in = len(in_names)
    bass_specs = (P("c"),) * (n_bass_in + len(out_names))
    stageB = jax.jit(
        shard_map(
            _bass_body, mesh=mesh, in_specs=bass_specs,
            out_specs=(P("c"),) * len(out_names), check_rep=False,
        ),
        keep_unused=True,
    )

    # ---- stage A: pure jax ----
    def _prep_body(xo, xa, wx, wh, bvec, ihx0, icx0, indr, lng, lnb):
        # xo, xa: [BL, s, 1024] bf16 (per core batch slice)
        # wx: [2048, 512] bf16 (per core gate-column slab), wh: [1024, 512]
        Wx = jax.lax.all_gather(wx, "c", axis=1, tiled=True)  # [2048, 4096]
        Wh = jax.lax.all_gather(wh, "c", axis=1, tiled=True)  # [1024, 4096]
        X2 = jnp.concatenate([xo, xa], axis=-1).reshape(BL * s_steps, INSZ)
        Z = (
            jnp.dot(X2, Wx, preferred_element_type=jnp.float32)
            + bvec[None, :]
        )
        zdl = (
            Z.reshape(BL, s_steps // ZCH, ZCH, 4, OSIZE)
            .astype(jnp.bfloat16)
            .transpose(1, 3, 0, 2, 4)
            .reshape(s_steps // ZCH, 4, BL, ZCH, OSIZE)
        )
        whpl = Wh.reshape(8, 128, NG).transpose(1, 0, 2)  # [128, 8, NG] bf16
        ihxT = jnp.broadcast_to(
            ihx0.reshape(8, 128).T[:, :, None], (128, 8, BL)
        ).astype(jnp.bfloat16)
        icxb = jnp.broadcast_to(icx0, (BL, OSIZE)).astype(jnp.float32)
        outs = [zdl, whpl, xo, ihxT, icxb, indr]
        if use_ln:
            l16g = jnp.repeat(lng, 32, axis=0)  # [4,1024]->[128,1024], row 32g+b
            l16b = jnp.repeat(lnb, 32, axis=0)
            outs += [l16g, l16b]
        outs.append(jnp.zeros((BL, s_steps, OSIZE), jnp.float32))  # out buffer
        return tuple(outs)

    a_in = (P("c"), P("c"), P(None, "c"), P(None, "c"), P(), P(), P(), P(), P(), P())
    a_out = (P("c"),) * (n_bass_in + 1)
    stageA = jax.jit(
        shard_map(_prep_body, mesh=mesh, in_specs=a_in, out_specs=a_out,
                  check_rep=False)
    )

    sh_b = NamedSharding(mesh, P("c"))
    sh_w = NamedSharding(mesh, P(None, "c"))
    sh_r = NamedSharding(mesh, P())

    def put(inputo, attn, W, bvec, ln_g, ln_b, init_hx, init_cx):
        import jax
        from concurrent.futures import ThreadPoolExecutor

        with ThreadPoolExecutor(3) as ex:
            fxo = ex.submit(lambda: _to_bf16(np.asarray(inputo)[:, :s_steps]))
            fxa = ex.submit(lambda: _to_bf16(np.asarray(attn)[:, :s_steps]))
            fwb = ex.submit(lambda: _to_bf16(np.asarray(W)))
            xo, xa, Wb = fxo.result(), fxa.result(), fwb.result()
        dev = dict(
            xo=jax.device_put(xo, sh_b),
            xa=jax.device_put(xa, sh_b),
            wx=jax.device_put(Wb[:INSZ], sh_w),
            wh=jax.device_put(Wb[INSZ:], sh_w),
            bvec=jax.device_put(np.asarray(bvec, np.float32), sh_r),
            ihx0=jax.device_put(
                np.asarray(init_hx, np.float32).reshape(OSIZE), sh_r
            ),
            icx0=jax.device_put(
                np.asarray(init_cx, np.float32).reshape(1, OSIZE), sh_r
            ),
            indr=jax.device_put(IND_NP, sh_r),
            lng=jax.device_put(np.asarray(ln_g, np.float32), sh_r),
            lnb=jax.device_put(np.asarray(ln_b, np.float32), sh_r),
        )
        return dev

    def run(dev):
        pre = stageA(
            dev["xo"], dev["xa"], dev["wx"], dev["wh"], dev["bvec"],
            dev["ihx0"], dev["icx0"], dev["indr"], dev["lng"], dev["lnb"],
        )
        outs = stageB(*pre)
        return outs[0]

    return run, put


def _get_pipeline(s_steps, use_ln):
    key = (s_steps, use_ln)
    if key not in _cache:
        _cache[key] = _build_pipeline(s_steps, use_ln)
    return _cache[key]


def kernel(inputo, attn, W, b, ln_g, ln_b, init_hx, init_cx):
    import jax

    ln_g = np.asarray(ln_g, np.float32)
    ln_b = np.asarray(ln_b, np.float32)
    use_ln = not (np.all(ln_g == 1.0) and np.all(ln_b == 0.0))
    run, put = _get_pipeline(S, use_ln)
    dev = put(inputo, attn, W, b, ln_g, ln_b, init_hx, init_cx)
    out = run(dev)
    return np.asarray(out)



# revision 5
# speedup vs baseline: 3.9767x; 3.9767x over previous
"""Trainium2 Bass kernel for nn_DecoderLayer (RNMT+ LN-LSTM decoder layer).

Two-stage pipeline, all device-resident between stages:

  stage A (jax/XLA, shard_map over 8 cores):
    - inputs uploaded as bf16 shards: X batch-sharded, W gate-column-sharded
      (8x less W upload than replication)
    - all_gather W columns, pre-phase GEMM  Z = [x,attn] @ Wx + b  (bf16, fp32
      accumulate), pack Z into the per-step layout the loop kernel wants
  stage B (Bass custom call, per core, batch-data-parallel BL=4):
    - the 256-step recurrence: g = z_t + h @ Wh; joint LayerNorm over the
      (4,1024) gate slab (bn_stats + indicator-matmul partition combine +
      fast-inverse-sqrt on DVE - no ACT table switches); sigmoid/tanh fused
      with the normalize via ACT scale/bias; c/h update; PE transpose of h
      for the next step's stationary operand; residual add + output.

Weights/activations bf16 (matmul), state and LN arithmetic fp32.
"""
import sys

sys.path.insert(0, "/opt/trn_rl_repo")

import numpy as np

import concourse.bass as bass
import concourse.tile as tile
from concourse import bacc, mybir

B, S, ISIZE, OSIZE = 32, 256, 1024, 1024
NCORES = 8
BL = B // NCORES  # 4 batch rows per core
INSZ = ISIZE + OSIZE  # 2048
NG = 4 * OSIZE  # 4096
EPS = 1e-5
F32, BF16, I32 = mybir.dt.float32, mybir.dt.bfloat16, mybir.dt.int32
FISR_MAGIC_F32 = float(
    np.frombuffer(np.array([0x5F3759DF], np.uint32).tobytes(), np.float32)[0]
)

ZCH = 4   # z prefetch chunk (steps per DMA)
RCH = 4   # res/out chunk (steps per DMA)

_cache = {}


def build_nc(s_steps=S, use_ln=False):
    assert s_steps % ZCH == 0 and s_steps % RCH == 0
    nc = bacc.Bacc(None)
    zd = nc.dram_tensor(
        "zd", [s_steps // ZCH, 4, BL, ZCH, OSIZE], BF16, kind="ExternalInput"
    )
    whp = nc.dram_tensor("whp", [128, 8, NG], BF16, kind="ExternalInput")
    res = nc.dram_tensor("res", [BL, s_steps, OSIZE], BF16, kind="ExternalInput")
    ihx = nc.dram_tensor("ihx", [128, 8, BL], BF16, kind="ExternalInput")
    icx = nc.dram_tensor("icx", [BL, OSIZE], F32, kind="ExternalInput")
    ind = nc.dram_tensor("ind", [128, 128], F32, kind="ExternalInput")
    if use_ln:
        lng = nc.dram_tensor("lng", [128, OSIZE], F32, kind="ExternalInput")
        lnb = nc.dram_tensor("lnb", [128, OSIZE], F32, kind="ExternalInput")
    out = nc.dram_tensor("out", [BL, s_steps, OSIZE], F32, kind="ExternalOutput")

    with tile.TileContext(nc) as tc:
        with (
            tc.tile_pool(name="cw", bufs=1) as cw,
            tc.tile_pool(name="rp", bufs=2) as rp,
            tc.tile_pool(name="ob", bufs=2) as ob,
            tc.tile_pool(name="sp", bufs=2) as sp,
            tc.tile_pool(name="hp", bufs=2) as hp,
            tc.tile_pool(name="yp", bufs=2) as yp,
            tc.tile_pool(name="htp", bufs=2) as htp,
            tc.tile_pool(name="gps", bufs=1, space="PSUM") as gps,
            tc.tile_pool(name="tps", bufs=1, space="PSUM") as tps,
            tc.tile_pool(name="yps", bufs=1, space="PSUM") as yps,
            tc.tile_pool(name="sps", bufs=1, space="PSUM") as sps,
        ):
            whs = cw.tile([128, 8, NG], BF16)
            nc.sync.dma_start(out=whs, in_=whp[:, :, :])
            inds = cw.tile([128, 128], F32)
            nc.sync.dma_start(out=inds, in_=ind[:, :])
            if use_ln:
                lngs = cw.tile([128, OSIZE], F32)
                nc.sync.dma_start(out=lngs, in_=lng[:, :])
                lnbs = cw.tile([128, OSIZE], F32)
                nc.sync.dma_start(out=lnbs, in_=lnb[:, :])
            magic = cw.tile([128, 1], F32)
            nc.vector.memset(magic, FISR_MAGIC_F32)
            id4 = cw.tile([BL, BL], F32)
            from concourse.masks import make_identity

            make_identity(nc, id4)

            c = cw.tile([BL, OSIZE], F32)
            nc.sync.dma_start(out=c, in_=icx[:, :])
            hT = htp.tile([128, 8, BL], BF16, tag="hT")
            nc.sync.dma_start(out=hT, in_=ihx[:, :, :])

            # persistent full-partition PSUM banks for the gate GEMM (one per
            # osize half); rows 32g+4..32g+31 stay 0 from this memset forever
            # (matmuls only ever write the BL valid rows of each col-group)
            psb = [gps.tile([128, 512], F32, name=f"psb{i}", tag=f"psb{i}") for i in range(2)]
            for p_ in psb:
                nc.vector.memset(p_, 0.0)
            # z double buffers: full 128 partitions, rows beyond the BL valid
            # ones per gate group stay 0 so full-width adds are safe
            z4bufs = [cw.tile([128, ZCH, OSIZE], BF16, name=f"z4b{i}", tag=f"z4b{i}") for i in range(2)]
            for zb in z4bufs:
                nc.vector.memset(zb, 0.0)

            for t in range(s_steps):
                tz, tr = t % ZCH, t % RCH
                if tz == 0:
                    z4 = z4bufs[(t // ZCH) % 2]
                    for zg in range(4):
                        nc.sync.dma_start(
                            out=z4[32 * zg : 32 * zg + BL, :, :],
                            in_=zd[t // ZCH, zg, :, :, :],
                        )
                if tr == 0:
                    resb = rp.tile([BL, RCH, OSIZE], BF16, tag="resb")
                    nc.gpsimd.dma_start(out=resb, in_=res[:, t : t + RCH, :])
                    outb = ob.tile([BL, RCH, OSIZE], F32, tag="outb")

                # ---- g = z_t + h @ Wh;  joint LN stats over the gate slab ----
                # 4-way column-tiled matmul: col-group g4 <-> gate g4, so the
                # four gates' GEMMs stream W concurrently on separate XBUSes
                # and land in the gsb partition layout directly.
                gsb = yp.tile([128, OSIZE], F32, tag="gsb")
                stats = sp.tile([128, 2, nc.vector.BN_STATS_DIM], F32, tag="stats")
                for h2 in range(2):
                    cs = slice(h2 * 512, h2 * 512 + 512)
                    pbank = psb[h2]
                    for kk in range(8):
                        for g4 in range(4):
                            nc.tensor.matmul(
                                pbank[32 * g4 : 32 * g4 + BL, :],
                                hT[:, kk, :],
                                whs[:, kk, g4 * 1024 + h2 * 512 : g4 * 1024 + h2 * 512 + 512],
                                start=(kk == 0),
                                stop=(kk == 7),
                                tile_position=(0, 32 * g4),
                                skip_group_check=True,
                            )
                    nc.vector.tensor_add(gsb[:, cs], pbank, z4[:, tz, cs])
                    nc.vector.bn_stats(
                        out=stats[:, h2, :], in_=gsb[:, h2 * 512 : h2 * 512 + 512]
                    )
                mv = sp.tile([128, 2], F32, tag="mv")
                nc.vector.bn_aggr(out=mv, in_=stats)
                # mv[:,1] <- E[x^2] = var + mean^2 (per partition)
                nc.vector.scalar_tensor_tensor(
                    out=mv[:, 1:2], in0=mv[:, 0:1], scalar=mv[:, 0:1],
                    in1=mv[:, 1:2],
                    op0=mybir.AluOpType.mult, op1=mybir.AluOpType.add,
                )
                pss = sps.tile([128, 2], F32, tag="pss")
                nc.tensor.matmul(pss, inds, mv, start=True, stop=True)
                q = sp.tile([128, 2], F32, tag="q")  # [mu, E[x^2]] per row
                nc.vector.tensor_copy(q, pss)
                vh = sp.tile([128, 1], F32, tag="vh")
                # vh = mu^2 - E[x^2]  (negated variance)
                nc.vector.scalar_tensor_tensor(
                    out=vh, in0=q[:, 0:1], scalar=q[:, 0:1], in1=q[:, 1:2],
                    op0=mybir.AluOpType.mult, op1=mybir.AluOpType.subtract,
                )
                # vh = -(vh) + EPS = var + EPS
                nc.vector.tensor_scalar(
                    out=vh, in0=vh, scalar1=-1.0, scalar2=EPS,
                    op0=mybir.AluOpType.mult, op1=mybir.AluOpType.add,
                )
                # fast inverse sqrt + 1 Newton iteration -> rstd
                ish = sp.tile([128, 1], I32, tag="ish")
                nc.vector.tensor_scalar(
                    out=ish, in0=vh.bitcast(I32), scalar1=1, scalar2=None,
                    op0=mybir.AluOpType.logical_shift_right,
                )
                y0 = sp.tile([128, 1], F32, tag="y0")
                nc.vector.tensor_sub(y0.bitcast(I32), magic.bitcast(I32), ish)
                t2 = sp.tile([128, 1], F32, tag="t2")
                nc.vector.scalar_tensor_tensor(
                    out=t2, in0=y0, scalar=y0, in1=vh,
                    op0=mybir.AluOpType.mult, op1=mybir.AluOpType.mult,
                )
                nc.vector.tensor_scalar(
                    out=t2, in0=t2, scalar1=-0.5, scalar2=1.5,
                    op0=mybir.AluOpType.mult, op1=mybir.AluOpType.add,
                )
                rstd = sp.tile([128, 1], F32, tag="rstd")
                nc.vector.tensor_mul(rstd, y0, t2)
                nbias = sp.tile([128, 1], F32, tag="nbias")  # -mu*rstd
                nc.vector.tensor_scalar(
                    out=nbias, in0=rstd, scalar1=q[:, 0:1], scalar2=-1.0,
                    op0=mybir.AluOpType.mult, op1=mybir.AluOpType.mult,
                )
                # activations: sigmoid gates -> PSUM (PSUM operands are
                # exempt from the equal-base-partition DVE rule), tanh -> SBUF
                ygs = yps.tile([96, OSIZE], F32, tag="ygs")
                ygt = yp.tile([BL, OSIZE], F32, tag="ygt")
                if use_ln:
                    y2n = yp.tile([128, OSIZE], F32, tag="y2n")
                    nc.vector.tensor_scalar(
                        out=y2n, in0=gsb, scalar1=rstd, scalar2=nbias,
                        op0=mybir.AluOpType.mult, op1=mybir.AluOpType.add,
                    )
                    nc.vector.tensor_mul(y2n, y2n, lngs)
                    nc.vector.tensor_add(y2n, y2n, lnbs)
                    nc.scalar.activation(
                        out=ygs, in_=y2n[0:96, :],
                        func=mybir.ActivationFunctionType.Sigmoid,
                    )
                    nc.scalar.activation(
                        out=ygt, in_=y2n[96 : 96 + BL, :],
                        func=mybir.ActivationFunctionType.Tanh,
                    )
                else:
                    nc.scalar.activation(
                        out=ygs, in_=gsb[0:96, :],
                        func=mybir.ActivationFunctionType.Sigmoid,
                        bias=nbias[0:96, :], scale=rstd[0:96, :],
                    )
                    nc.scalar.activation(
                        out=ygt, in_=gsb[96 : 96 + BL, :],
                        func=mybir.ActivationFunctionType.Tanh,
                        bias=nbias[96 : 96 + BL, :],
                        scale=rstd[96 : 96 + BL, :],
                    )

                # keep-warm: tiny PE op mid-tail so HAM stays at full clock
                psT = tps.tile([128, 8, BL], F32, tag="psT")
                nc.tensor.transpose(psT[0:64, 0, :], ygt[:, 0:64], id4)

                # ---- state update ----
                u = hp.tile([BL, OSIZE], F32, tag="u")
                nc.vector.tensor_mul(u, ygs[0:BL, :], ygt)
                # second keep-warm, fires after u mid/late tail
                nc.tensor.transpose(psT[0:64, 1, :], u[:, 0:64], id4)
                nc.vector.tensor_mul(c, ygs[32 : 32 + BL, :], c)
                nc.vector.tensor_add(c, c, u)
                h = hp.tile([BL, OSIZE], F32, tag="h")
                for hh in range(2):
                    cs = slice(hh * 512, hh * 512 + 512)
                    nc.vector.tensor_mul(h[:, cs], ygs[64 : 64 + BL, cs], c[:, cs])
                    if t + 1 < s_steps:
                        for kk in range(4 * hh, 4 * hh + 4):
                            nc.tensor.transpose(
                                psT[:, kk, :], h[:, kk * 128 : (kk + 1) * 128], id4
                            )
                nc.gpsimd.tensor_add(outb[:, tr, :], h, resb[:, tr, :])
                if tr == RCH - 1:
                    nc.gpsimd.dma_start(
                        out=out[:, t - tr : t + 1, :], in_=outb[:, :, :]
                    )
                if t + 1 < s_steps:
                    hT = htp.tile([128, 8, BL], BF16, tag="hT")
                    nc.vector.tensor_copy(hT, psT)
    nc.finalize()
    return nc



# ---------------------------------------------------------------------------
# host prep + jax pipeline (stage A: gathers + pre-phase GEMM; stage B: bass)
# ---------------------------------------------------------------------------
import ml_dtypes

BF16NP = ml_dtypes.bfloat16


def _to_bf16(a):
    """fp32 -> bf16 with round-to-nearest-even via integer view (fast)."""
    u = np.ascontiguousarray(a, np.float32).view(np.uint32)
    r = ((u + np.uint32(0x7FFF) + ((u >> np.uint32(16)) & np.uint32(1)))
         >> np.uint32(16)).astype(np.uint16)
    return r.view(BF16NP)


_pp = np.arange(128)
IND_NP = 0.25 * (
    (_pp[:, None] % 32 == _pp[None, :] % 32) & (_pp[:, None] % 32 < 4)
).astype(np.float32)


def _build_pipeline(s_steps, use_ln):
    """Returns (run, put) where put(host arrays)->device arrays and
    run(dev)->jax out array [B, s, OSIZE]."""
    import jax
    import jax.numpy as jnp
    from jax.sharding import Mesh, PartitionSpec as P, NamedSharding
    from jax.experimental.shard_map import shard_map
    from concourse.bass2jax import (
        install_neuronx_cc_hook,
        partition_id_tensor,
        _bass_exec_p,
    )

    install_neuronx_cc_hook()
    nc = build_nc(s_steps, use_ln)

    devices = jax.devices()[:NCORES]
    mesh = Mesh(np.asarray(devices), ("c",))

    # ---- stage B: bass custom call ----
    partition_name = nc.partition_id_tensor.name if nc.partition_id_tensor else None
    in_names, out_names, out_avals = [], [], []
    for alloc in nc.m.functions[0].allocations:
        if not isinstance(alloc, mybir.MemoryLocationSet):
            continue
        name = alloc.memorylocations[0].name
        if alloc.kind == "ExternalInput":
            if name != partition_name:
                in_names.append(name)
        elif alloc.kind == "ExternalOutput":
            out_names.append(name)
            import jax.core

            out_avals.append(
                jax.core.ShapedArray(tuple(alloc.tensor_shape), mybir.dt.np(alloc.dtype))
            )
    all_in = in_names + out_names + ([partition_name] if partition_name else [])

    def _bass_body(*args):
        operands = list(args)
        if partition_name is not None:
            operands.append(partition_id_tensor())
        outs = _bass_exec_p.bind(
            *operands,
            out_avals=tuple(out_avals),
            in_names=tuple(all_in),
            out_names=tuple(out_names),
            lowering_input_output_aliases=(),
            sim_require_finite=True,
            sim_require_nnan=True,
            nc=nc,
        )
        return tuple(outs)

    # bass input order: zd, whp, res, ihx, icx, ind, [lng, lnb] then out-zeros
    n_bass_in = len(in_names)
    bass_specs = (P("c"),) * (n_bass_in + len(out_names))
    stageB = jax.jit(
        shard_map(
            _bass_body, mesh=mesh, in_specs=bass_specs,
            out_specs=(P("c"),) * len(out_names), check_rep=False,
        ),
        keep_unused=True,
    )

    # ---- stage A: pure jax ----
    def _prep_body(xo, xa, wx, wh, bvec, ihx0, icx0, indr, lng, lnb):
        # xo, xa: [BL, s, 1024] bf16 (per core batch slice)
        # wx: [2048, 512] bf16 (per core gate-column slab), wh: [1024, 512]
        Wx = jax.lax.all_gather(wx, "c", axis=1, tiled=True)  # [2048, 4096]
        Wh = jax.lax.all_gather(wh, "c", axis=1, tiled=True)  # [1024, 4096]
        X2 = jnp.concatenate([xo, xa], axis=-1).reshape(BL * s_steps, INSZ)
        Z = (
            jnp.dot(X2, Wx, preferred_element_type=jnp.float32)
            + bvec[None, :]
        )
        zdl = (
            Z.reshape(BL, s_steps // ZCH, ZCH, 4, OSIZE)
            .astype(jnp.bfloat16)
            .transpose(1, 3, 0, 2, 4)
            .reshape(s_steps // ZCH, 4, BL, ZCH, OSIZE)
        )
        whpl = Wh.reshape(8, 128, NG).transpose(1, 0, 2)  # [128, 8, NG] bf16
        ihxT = jnp.broadcast_to(
            ihx0.reshape(8, 128).T[:, :, None], (128, 8, BL)
        ).astype(jnp.bfloat16)
        icxb = jnp.broadcast_to(icx0, (BL, OSIZE)).astype(jnp.float32)
        outs = [zdl, whpl, xo, ihxT, icxb, indr]
        if use_ln:
            l16g = jnp.repeat(lng, 32, axis=0)  # [4,1024]->[128,1024], row 32g+b
            l16b = jnp.repeat(lnb, 32, axis=0)
            outs += [l16g, l16b]
        outs.append(jnp.zeros((BL, s_steps, OSIZE), jnp.float32))  # out buffer
        return tuple(outs)

    a_in = (P("c"), P("c"), P(None, "c"), P(None, "c"), P(), P(), P(), P(), P(), P())
    a_out = (P("c"),) * (n_bass_in + 1)
    stageA = jax.jit(
        shard_map(_prep_body, mesh=mesh, in_specs=a_in, out_specs=a_out,
                  check_rep=False)
    )

    sh_b = NamedSharding(mesh, P("c"))
    sh_w = NamedSharding(mesh, P(None, "c"))
    sh_r = NamedSharding(mesh, P())

    def put(inputo, attn, W, bvec, ln_g, ln_b, init_hx, init_cx):
        import jax
        from concurrent.futures import ThreadPoolExecutor

        with ThreadPoolExecutor(3) as ex:
            fxo = ex.submit(lambda: _to_bf16(np.asarray(inputo)[:, :s_steps]))
            fxa = ex.submit(lambda: _to_bf16(np.asarray(attn)[:, :s_steps]))
            fwb = ex.submit(lambda: _to_bf16(np.asarray(W)))
            xo, xa, Wb = fxo.result(), fxa.result(), fwb.result()
        dev = dict(
            xo=jax.device_put(xo, sh_b),
            xa=jax.device_put(xa, sh_b),
            wx=jax.device_put(Wb[:INSZ], sh_w),
            wh=jax.device_put(Wb[INSZ:], sh_w),
            bvec=jax.device_put(np.asarray(bvec, np.float32), sh_r),
            ihx0=jax.device_put(
                np.asarray(init_hx, np.float32).reshape(OSIZE), sh_r
            ),
            icx0=jax.device_put(
                np.asarray(init_cx, np.float32).reshape(1, OSIZE), sh_r
            ),
            indr=jax.device_put(IND_NP, sh_r),
            lng=jax.device_put(np.asarray(ln_g, np.float32), sh_r),
            lnb=jax.device_put(np.asarray(ln_b, np.float32), sh_r),
        )
        return dev

    def run(dev):
        pre = stageA(
            dev["xo"], dev["xa"], dev["wx"], dev["wh"], dev["bvec"],
            dev["ihx0"], dev["icx0"], dev["indr"], dev["lng"], dev["lnb"],
        )
        outs = stageB(*pre)
        return outs[0]

    _dbg[(s_steps, use_ln)] = dict(stageA=stageA, stageB=stageB, nc=nc)
    return run, put


_dbg = {}


def _get_pipeline(s_steps, use_ln):
    key = (s_steps, use_ln)
    if key not in _cache:
        _cache[key] = _build_pipeline(s_steps, use_ln)
    return _cache[key]


def kernel(inputo, attn, W, b, ln_g, ln_b, init_hx, init_cx):
    import jax

    ln_g = np.asarray(ln_g, np.float32)
    ln_b = np.asarray(ln_b, np.float32)
    use_ln = not (np.all(ln_g == 1.0) and np.all(ln_b == 0.0))
    run, put = _get_pipeline(S, use_ln)
    dev = put(inputo, attn, W, b, ln_g, ln_b, init_hx, init_cx)
    out = run(dev)
    return np.asarray(out)

